# revision 16
# baseline (speedup 1.0000x reference)
"""Distributed Trainium2 attention kernel (8 NeuronCores).

Strategy: tensor-parallel over heads for QKV projection + attention
(4 query heads + their 1 shared KV head per core, identical causal loop
structure on every core), then AllToAlls switch to row-sharding so each
core computes the output projection for its 512 rows with the full wo.
Host reassembles rows. All matmuls run in bf16 with fp32 PSUM
accumulation; softmax runs unnormalized with the normalization folded in
after the PV matmul (per-head row sums via a ones-matmul).

RoPE is applied in row-major layout via a host-side even/odd column
permutation of wq/wk (rotation becomes contiguous half-block arithmetic),
then q/k are transposed to [head_dim, rows] on the TensorEngine for the
attention matmuls.

Perf structure (the tile scheduler overlaps phases wherever data deps
allow, so emission order mostly sets priorities):
- x / wo / rope tables are host-packed so SBUF tiles fill from 1-2
  contiguous-slab DMAs split across engine queues (per-queue DMA
  bandwidth is only ~90GB/s, and per-tile dma_start issue costs ~630ns).
- QKV weights stream on the scalar queue so x slabs never sit behind them.
- The attention worklist is HEAD-major and each head's AllToAll fires as
  soon as that head's 8 chains are done, so all comm except the last
  1MB AllToAll overlaps attention; the output projection is chunked
  per-head with an fp32 SBUF accumulator so the last AllToAll hides
  under the first chunks' matmuls.
- The softmax normalization tail uses reciprocal_approx_fast (0.7us vs
  3.3us exact), and PSUM is split into per-role rings so a chain's
  scores never wait on the previous chain's tail.
"""

import numpy as np
import ml_dtypes
from contextlib import ExitStack

import concourse.bass as bass
import concourse.mybir as mybir
import concourse.tile as tile
from concourse import bacc
from concourse import bass_utils

B, S, D = 2, 2048, 4096
H, HKV, HD = 32, 8, 128
HD2 = HD // 2
NC = 8
HL = H // NC            # 4 local q heads per core
BS = B * S              # 4096 global rows
R = BS // NC            # 512 output rows per core
NRB = BS // 128         # 32 row blocks
NDT = D // 128          # 32 contraction tiles
SCALE = 1.0 / float(np.sqrt(HD))
BF = mybir.dt.bfloat16
F32 = mybir.dt.float32

PROFILE = False         # set by test.py for neuron-profile capture
TMPDIR = None           # set by test.py to keep the trace dir


def _emit(nc, tc, io):
    xTr, wqkvT, woTr, ccssR, trim, onec, iden, onerow, out = io

    cstack = ExitStack()
    bstack = ExitStack()
    with (
        tc.tile_pool(name="ps", bufs=1, space="PSUM") as ps,
        tc.tile_pool(name="cbuf", bufs=1) as cbuf,
        tc.tile_pool(name="cs", bufs=3) as cs,
        tc.tile_pool(name="es", bufs=6) as es,
        tc.tile_pool(name="ts", bufs=8) as ts,
        tc.tile_pool(name="ans", bufs=4) as ans,
        tc.tile_pool(name="rsp", bufs=3) as rsp,
        tc.tile_pool(name="dram", bufs=1, space="DRAM") as dram,
    ):
        # pools released after phase C (attention) to make room for the
        # fp32 output accumulator.
        qbuf = cstack.enter_context(tc.tile_pool(name="qbuf", bufs=1))
        kvbuf = cstack.enter_context(tc.tile_pool(name="kvbuf", bufs=1))
        # pools released after phase B (projection).
        wbuf = bstack.enter_context(tc.tile_pool(name="wbuf", bufs=1))
        xs = bstack.enter_context(tc.tile_pool(name="xs", bufs=3))

        q_sb = qbuf.tile([128, HL * BS], BF, tag="q")     # col = h*4096 + row
        kT_sb = kvbuf.tile([128, BS], BF, tag="k")        # col = row
        v_sb = kvbuf.tile([128, BS], BF, tag="v")         # col = rb*128 + hd

        # x slabs: host-packed so slab rb = [128, 32*128] contiguous; split
        # into two half-slab DMAs on different engines (per-queue DMA BW).
        xt_tiles = {}
        ccss_tiles = {}

        def load_rb(rb):
            xt = xs.tile([128, NDT * 128], BF, tag="x", name=f"xt{rb}")
            nc.sync.dma_start(
                xt[:, 0:2048], xTr[rb * 128: (rb + 1) * 128, 0:2048])
            nc.gpsimd.dma_start(
                xt[:, 2048:4096], xTr[rb * 128: (rb + 1) * 128, 2048:4096])
            xt_tiles[rb] = xt
            cst = cs.tile([128, 512], BF, tag="cc", name=f"cs{rb}")
            nc.gpsimd.dma_start(cst[:], ccssR[:, rb * 512: (rb + 1) * 512])
            ccss_tiles[rb] = cst

        load_rb(0)
        load_rb(1)

        # constants (sync queue, tiny)
        trim_sb = cbuf.tile([128, 128], F32, tag="tm")
        nc.sync.dma_start(trim_sb[:], trim[:])
        onec_sb = cbuf.tile([128, 1], BF, tag="oc")
        nc.sync.dma_start(onec_sb[:], onec[:])
        iden_sb = cbuf.tile([128, 128], BF, tag="idn")
        nc.sync.dma_start(iden_sb[:], iden[:])
        onerow_sb = cbuf.tile([1, 128], F32, tag="orw")
        nc.sync.dma_start(onerow_sb[:], onerow[:])

        # resident QKV weights: col = dt*768 + [0:512 q | 512:640 k | 640:768 v]
        # all on the scalar queue (idle during early phase B).
        w_sb = wbuf.tile([128, NDT * 768], BF, tag="w")
        for dt in range(NDT):
            nc.scalar.dma_start(
                w_sb[:, dt * 768: dt * 768 + 768],
                wqkvT[dt * 128: (dt + 1) * 128, :],
            )

        # Per-head AllToAll buffers: input row block (b*4+ci)*128+hd holds
        # this core's head h attention output for destination rank b*4+ci;
        # output row block i*128+hd holds source core i's head (4i+h) for
        # this core's rows.
        a2a_in = [dram.tile([8 * 128, R], BF, name=f"a2a_in{h}")
                  for h in range(HL)]
        a2a_out = [dram.tile([8 * 128, R], BF, name=f"a2a_out{h}")
                   for h in range(HL)]

        # ---- phase B: QKV projection + RoPE + transposes ----
        def b_rope_tail_q(rb, ps_q):
            cct = ccss_tiles[rb][:, 0:256]
            sst = ccss_tiles[rb][:, 256:512]
            # q rotation, all 4 heads at once via strided APs
            qe = ps_q[:].rearrange("p (h d) -> p h d", d=128)[:, :, 0:HD2]
            qo = ps_q[:].rearrange("p (h d) -> p h d", d=128)[:, :, HD2:HD]
            t1 = ts.tile([128, 256], BF, tag="t")
            t2 = ts.tile([128, 256], BF, tag="t")
            t3 = ts.tile([128, 256], BF, tag="t")
            t4 = ts.tile([128, 256], BF, tag="t")
            nc.vector.tensor_mul(t1[:], qe, cct)
            nc.vector.tensor_mul(t2[:], qo, sst)
            nc.vector.tensor_mul(t3[:], qe, sst)
            nc.vector.tensor_mul(t4[:], qo, cct)
            qrot = ts.tile([128, 512], BF, tag="qr", bufs=4)
            qre = qrot[:].rearrange("p (h d) -> p h d", d=128)[:, :, 0:HD2]
            qro = qrot[:].rearrange("p (h d) -> p h d", d=128)[:, :, HD2:HD]
            nc.vector.tensor_sub(qre, t1[:], t2[:])
            nc.vector.tensor_add(qro, t3[:], t4[:])
            return (qrot,)

        def b_transpose_tail_q(rb, qrot):
            ps_tq = ps.tile([128, 512], BF, tag="at", bufs=2)
            for h in range(HL):
                nc.tensor.transpose(
                    ps_tq[:, h * 128: (h + 1) * 128],
                    qrot[:, h * 128: (h + 1) * 128],
                    iden_sb[:],
                )
            q_dst = (
                q_sb[:]
                .rearrange("p (h r) -> p h r", h=HL)
                [:, :, rb * 128: (rb + 1) * 128]
            )
            nc.vector.tensor_copy(
                q_dst, ps_tq[:].rearrange("p (h r) -> p h r", h=HL)
            )

        def b_rope_tail_kv(rb, ps_kv):
            cct = ccss_tiles[rb][:, 0:256]
            sst = ccss_tiles[rb][:, 256:512]
            ke = ps_kv[:, 0:HD2]
            ko = ps_kv[:, HD2:HD]
            u1 = ts.tile([128, 64], BF, tag="u")
            u2 = ts.tile([128, 64], BF, tag="u")
            u3 = ts.tile([128, 64], BF, tag="u")
            u4 = ts.tile([128, 64], BF, tag="u")
            nc.vector.tensor_mul(u1[:], ke, cct[:, 0:HD2])
            nc.vector.tensor_mul(u2[:], ko, sst[:, 0:HD2])
            nc.vector.tensor_mul(u3[:], ke, sst[:, 0:HD2])
            nc.vector.tensor_mul(u4[:], ko, cct[:, 0:HD2])
            krot = ts.tile([128, 128], BF, tag="kr")
            nc.vector.tensor_sub(krot[:, 0:HD2], u1[:], u2[:])
            nc.vector.tensor_add(krot[:, HD2:HD], u3[:], u4[:])

            # v: plain copy to row-major storage
            nc.scalar.activation(
                v_sb[:, rb * 128: (rb + 1) * 128], ps_kv[:, 128:256],
                mybir.ActivationFunctionType.Copy,
            )
            return (krot,)

        def b_transpose_tail_kv(rb, krot):
            ps_tk = ps.tile([128, 128], BF, tag="rs", bufs=2)
            nc.tensor.transpose(ps_tk[:], krot[:], iden_sb[:])
            nc.vector.tensor_copy(kT_sb[:, rb * 128: (rb + 1) * 128], ps_tk[:])

        pending = None
        rot = None
        for rb in range(NRB):
            if rb + 2 < NRB:
                load_rb(rb + 2)
            ps_q = ps.tile([128, 512], F32, tag="s", bufs=4)  # [rows, 4 q heads]
            ps_kv = ps.tile([128, 256], F32, tag="s", bufs=4)  # [rows, k|v]
            xt = xt_tiles[rb]
            for dt in range(NDT):
                st, sp = dt == 0, dt == NDT - 1
                nc.tensor.matmul(
                    ps_q[:], xt[:, dt * 128: (dt + 1) * 128],
                    w_sb[:, dt * 768: dt * 768 + 512],
                    start=st, stop=sp,
                )
                nc.tensor.matmul(
                    ps_kv[:], xt[:, dt * 128: (dt + 1) * 128],
                    w_sb[:, dt * 768 + 512: dt * 768 + 768],
                    start=st, stop=sp,
                )
                if dt == 2 and pending is not None:
                    rot = (pending[0],) + b_rope_tail_q(pending[0], pending[1]) \
                        + b_rope_tail_kv(pending[0], pending[2])
                    pending = None
                if dt == 12 and rot is not None:
                    b_transpose_tail_q(rot[0], rot[1])
                    b_transpose_tail_kv(rot[0], rot[2])
                    rot = None
            pending = (rb, ps_q, ps_kv)
            del xt_tiles[rb]
        rot = (pending[0],) + b_rope_tail_q(pending[0], pending[1]) \
            + b_rope_tail_kv(pending[0], pending[2])
        b_transpose_tail_q(rot[0], rot[1])
        b_transpose_tail_kv(rot[0], rot[2])

        # release B-only SBUF (weights + x slabs); open D-phase streaming
        # pools on the right side of SBUF.
        bstack.close()
        abuf = tc.alloc_tile_pool(name="abuf", bufs=2, side="right")
        ws = tc.alloc_tile_pool(name="ws", bufs=2, side="right")
        osp = tc.alloc_tile_pool(name="os", bufs=3, side="right")

        # ---- phase C: causal attention, paired interleaved chains ----
        # Each (b, h, ci) is an independent chain; two chains are emitted
        # interleaved so one chain's exp latency hides under the other's
        # matmuls. Pairing ci=0 with ci=3 (and 1 with 2) balances lengths.
        def attn_chain(b, h, ci):
            qbase = h * BS + b * S
            ps_attn = ps.tile([128, 512], F32, tag="at", bufs=2,
                              name=f"pa{b}{h}{ci}")
            ps_rs = ps.tile([1, 512], F32, tag="rs", bufs=2,
                            name=f"pr{b}{h}{ci}")
            jmax = 4 * ci + 3

            def qspan(j):
                q0 = max(j * 128, 512 * ci)
                return q0, 512 * ci + 512 - q0

            def scores(j):
                q0, w = qspan(j)
                kcol = (b * 16 + j) * 128
                ps_s = ps.tile([128, 512], F32, tag="s", bufs=4, name=f"s{j}")
                nc.tensor.matmul(
                    ps_s[:, 0:w],
                    kT_sb[:, kcol: kcol + 128],
                    q_sb[:, qbase + q0: qbase + q0 + w],
                    start=True, stop=True,
                )
                if j // 4 == ci:
                    nc.vector.tensor_add(
                        ps_s[:, 0:128], ps_s[:, 0:128], trim_sb[:]
                    )
                et = es.tile([128, 512], BF, tag="e", name=f"e{j}")
                nc.scalar.activation(
                    et[:, 0:w], ps_s[:, 0:w],
                    mybir.ActivationFunctionType.Exp, scale=SCALE,
                )
                return et

            def pv(j, et):
                q0, w = qspan(j)
                off = q0 - 512 * ci
                kcol = (b * 16 + j) * 128
                nc.tensor.matmul(
                    ps_attn[:, off: off + w],
                    v_sb[:, kcol: kcol + 128],
                    et[:, 0:w],
                    start=(j == 0), stop=(j == jmax),
                )
                nc.tensor.matmul(
                    ps_rs[:, off: off + w],
                    onec_sb[:],
                    et[:, 0:w],
                    start=(j == 0), stop=(j == jmax),
                )

            prev = None
            for j in range(jmax + 1):
                et = scores(j)
                if prev is not None:
                    pv(prev[0], prev[1])
                prev = (j, et)
                yield
            pv(prev[0], prev[1])
            # Tail is spread across engines so the PSUM rings release fast
            # (they pace the next chains): DVE does only the reciprocal,
            # ScalarE evacuates the unnormalized attn (frees the 'at' bank),
            # gpsimd broadcasts and multiplies in SBUF.
            rc = rsp.tile([1, 512], F32, tag="rc")
            nc.vector.reciprocal_approx_fast(rc[:], ps_rs[:])
            araw = ans.tile([128, 512], BF, tag="ar")
            nc.scalar.activation(araw[:], ps_attn[:],
                                 mybir.ActivationFunctionType.Copy)
            bc_sb = rsp.tile([128, 512], F32, tag="bcs")
            nc.gpsimd.partition_broadcast(bc_sb[:], rc[:])
            an = ans.tile([128, 512], BF, tag="an")
            nc.gpsimd.tensor_mul(an[:], araw[:], bc_sb[:])
            blk = (b * 4 + ci) * 128
            nc.sync.dma_start(a2a_in[h][blk: blk + 128, :], an[:])
            yield

        # Head-major worklist; fire head h's AllToAll as soon as its 8
        # chains are done so only the last AllToAll lands after attention.
        def drive(todo):
            todo = list(todo)
            active = []
            while todo or active:
                while len(active) < 2 and todo:
                    active.append(attn_chain(*todo.pop(0)))
                for g in list(active):
                    if next(g, StopIteration) is StopIteration:
                        active.remove(g)

        for h in range(HL):
            drive([(b, h, ci) for b in range(B) for ci in (0, 3, 1, 2)])
            nc.gpsimd.collective_compute(
                "AllToAll",
                mybir.AluOpType.bypass,
                replica_groups=[list(range(NC))],
                ins=[a2a_in[h].opt()],
                outs=[a2a_out[h].opt()],
            )

        # attention buffers dead; reuse their SBUF for the fp32 output
        # accumulator (chunked output projection).
        cstack.close()
        accbuf = tc.alloc_tile_pool(name="accbuf", bufs=1)
        acc = [accbuf.tile([128, D], F32, tag=f"a{rt}", name=f"acc{rt}")
               for rt in range(4)]

        # ---- phase D: output projection, one chunk per head index ----
        # chunk k covers head-tiles ht = 4i+k (head k of each source core);
        # wo slabs are host-packed so slab (k, cg) = [128, 8*512] contiguous.
        for k in range(4):
            at_k = abuf.tile([128, 8 * 512], BF, tag="at", name=f"at{k}")
            nc.gpsimd.dma_start(
                at_k[:].rearrange("p (i c) -> p i c", i=8),
                a2a_out[k][:].rearrange("(i p) c -> p i c", p=128))
            wt = None
            nxt = ws.tile([128, 8 * 512], BF, tag="wo", name=f"wt{k}0")
            nc.scalar.dma_start(
                nxt[:, 0:2048], woTr[(k * 8) * 128: (k * 8 + 1) * 128, 0:2048])
            nc.sync.dma_start(
                nxt[:, 2048:4096],
                woTr[(k * 8) * 128: (k * 8 + 1) * 128, 2048:4096])
            for cg in range(8):
                wt, nxt = nxt, None
                if cg < 7:
                    row = (k * 8 + cg + 1) * 128
                    nxt = ws.tile([128, 8 * 512], BF, tag="wo",
                                  name=f"wt{k}{cg + 1}")
                    nc.scalar.dma_start(nxt[:, 0:2048], woTr[row: row + 128, 0:2048])
                    nc.sync.dma_start(
                        nxt[:, 2048:4096], woTr[row: row + 128, 2048:4096])
                for rt in range(4):
                    po = ps.tile([128, 512], F32, tag="s", bufs=4,
                                 name=f"po{k}{cg}{rt}")
                    for i in range(8):
                        nc.tensor.matmul(
                            po[:],
                            at_k[:, i * 512 + rt * 128: i * 512 + rt * 128 + 128],
                            wt[:, i * 512: (i + 1) * 512],
                            start=(i == 0), stop=(i == 7),
                        )
                    aslice = acc[rt][:, cg * 512: (cg + 1) * 512]
                    if k == 0:
                        nc.vector.tensor_copy(aslice, po[:])
                    elif k < 3:
                        nc.vector.tensor_add(aslice, aslice, po[:])
                    else:
                        ot = osp.tile([128, 512], F32, tag="o")
                        nc.vector.tensor_add(ot[:], aslice, po[:])
                        nc.sync.dma_start(
                            out[rt * 128: (rt + 1) * 128,
                                cg * 512: (cg + 1) * 512], ot[:])
        accbuf.release()
        osp.release()
        ws.release()
        abuf.release()


def _build():
    # NOTE: enable-ldw-opt=true crashes walrus codegen in visitInstLdweights;
    # do not enable.
    nc = bacc.Bacc("TRN2", target_bir_lowering=False, debug=False, num_devices=NC)
    xTr = nc.dram_tensor("xTr", [BS, D], BF, kind="ExternalInput")
    wqkvT = nc.dram_tensor("wqkvT", [D, 768], BF, kind="ExternalInput")
    woTr = nc.dram_tensor("woTr", [D, D], BF, kind="ExternalInput")
    ccssR = nc.dram_tensor("ccssR", [128, NRB * 512], BF, kind="ExternalInput")
    trim = nc.dram_tensor("trim", [128, 128], F32, kind="ExternalInput")
    onec = nc.dram_tensor("onec", [128, 1], BF, kind="ExternalInput")
    iden = nc.dram_tensor("iden", [128, 128], BF, kind="ExternalInput")
    onerow = nc.dram_tensor("onerow", [1, 128], F32, kind="ExternalInput")
    out = nc.dram_tensor("out", [R, D], F32, kind="ExternalOutput")
    with tile.TileContext(nc) as tc:
        _emit(nc, tc, (xTr, wqkvT, woTr, ccssR, trim, onec, iden, onerow, out))
    nc.compile()
    return nc


_NC = None


def kernel(x, wq, wk, wv, wo, freqs_cos, freqs_sin, mask, start_pos):
    global _NC
    if _NC is None:
        _NC = _build()
    nc = _NC
    bf = ml_dtypes.bfloat16

    x = np.asarray(x, dtype=np.float32)
    xT = np.ascontiguousarray(x.reshape(BS, D).T)
    # pack so slab rb = [128 partitions, 32 dt * 128 cols] is contiguous:
    # xTr[rb*128 + p, dt*128 + c] = xT[dt*128 + p, rb*128 + c]
    xTr = np.ascontiguousarray(
        xT.reshape(NDT, 128, NRB, 128).transpose(2, 1, 0, 3).reshape(BS, D)
    ).astype(bf)

    perm = np.concatenate([np.arange(0, HD, 2), np.arange(1, HD, 2)])
    wqTp = np.asarray(wq, np.float32).T.reshape(D, H, HD)[:, :, perm]
    wkTp = np.asarray(wk, np.float32).T.reshape(D, HKV, HD)[:, :, perm]
    wvT = np.asarray(wv, np.float32).T.reshape(D, HKV, HD)
    woT = np.asarray(wo, np.float32).T
    # pack so slab (k, cg) = [128 partitions, 8 i * 512 cols] is contiguous:
    # woTr[(k*8+cg)*128 + p, i*512 + c] = woT[(i*4+k)*128 + p, cg*512 + c]
    woTr = np.ascontiguousarray(
        woT.reshape(8, 4, 128, 8, 512).transpose(1, 3, 2, 0, 4).reshape(D, D)
    ).astype(bf)

    fc = np.asarray(freqs_cos, np.float32)
    fs = np.asarray(freqs_sin, np.float32)
    # row-major RoPE tables per row block, replicated x4 along free axis,
    # cos and sin packed side by side: [128, rb*512 + (0:256 cos|256:512 sin)]
    pos = (np.arange(BS) % S).reshape(NRB, 128)
    ccR = np.tile(fc[pos], (1, 1, 4))          # (NRB, 128, 256)
    ssR = np.tile(fs[pos], (1, 1, 4))
    ccssR = np.concatenate([ccR, ssR], axis=2)  # (NRB, 128, 512)
    ccssR = np.ascontiguousarray(
        ccssR.transpose(1, 0, 2).reshape(128, NRB * 512)
    ).astype(bf)

    trim = np.where(
        np.arange(128)[:, None] > np.arange(128)[None, :], -1e30, 0.0
    ).astype(np.float32)
    onec = np.ones((128, 1), dtype=bf)
    iden = np.eye(128, dtype=bf)
    onerow = np.ones((1, 128), dtype=np.float32)

    in_maps = []
    for c in range(NC):
        wqkv = np.concatenate(
            [
                wqTp[:, 4 * c: 4 * c + 4].reshape(D, 512),
                wkTp[:, c],
                wvT[:, c],
            ],
            axis=1,
        ).astype(bf)
        in_maps.append(
            {
                "xTr": xTr,
                "wqkvT": np.ascontiguousarray(wqkv),
                "woTr": woTr,
                "ccssR": ccssR,
                "trim": trim,
                "onec": onec,
                "iden": iden,
                "onerow": onerow,
            }
        )

    res = bass_utils.run_bass_kernel_spmd(
        nc, in_maps, core_ids=list(range(NC)), trace=PROFILE, tmpdir=TMPDIR
    )
    if PROFILE:
        print(f"HW exec time: {res.exec_time_ns} ns")
        if res.instructions_and_trace is not None:
            print(f"trace: {res.instructions_and_trace[1]}")

    out_full = np.empty((BS, D), dtype=np.float32)
    for c in range(NC):
        out_full[R * c: R * (c + 1)] = res.results[c]["out"]
    return out_full.reshape(B, S, D)


# revision 17
# speedup vs baseline: 1.1967x; 1.1967x over previous
"""Distributed Trainium2 attention kernel (8 NeuronCores).

Strategy: tensor-parallel over heads for QKV projection + attention
(4 query heads + their 1 shared KV head per core, identical causal loop
structure on every core), then AllToAlls switch to row-sharding so each
core computes the output projection for its 512 rows with the full wo.
Host reassembles rows. All matmuls run in bf16 with fp32 PSUM
accumulation; softmax runs unnormalized with the normalization folded in
after the PV matmul (per-head row sums via a ones-matmul).

RoPE is applied in row-major layout via a host-side even/odd column
permutation of wq/wk (rotation becomes contiguous half-block arithmetic),
then q/k are transposed to [head_dim, rows] on the TensorEngine for the
attention matmuls.

Perf structure (the tile scheduler overlaps phases wherever data deps
allow, so emission order mostly sets priorities):
- x / wo / rope tables are host-packed so SBUF tiles fill from 1-2
  contiguous-slab DMAs split across engine queues (per-queue DMA
  bandwidth is only ~90GB/s, and per-tile dma_start issue costs ~630ns).
- QKV weights stream on the scalar queue so x slabs never sit behind them.
- The attention worklist is HEAD-major and each head's AllToAll fires as
  soon as that head's 8 chains are done, so all comm except the last
  1MB AllToAll overlaps attention; the output projection is chunked
  per-head with an fp32 SBUF accumulator so the last AllToAll hides
  under the first chunks' matmuls.
- The softmax normalization tail uses reciprocal_approx_fast (0.7us vs
  3.3us exact), and PSUM is split into per-role rings so a chain's
  scores never wait on the previous chain's tail.
"""

import numpy as np
import ml_dtypes
from contextlib import ExitStack

import concourse.bass as bass
import concourse.mybir as mybir
import concourse.tile as tile
from concourse import bacc
from concourse import bass_utils

B, S, D = 2, 2048, 4096
H, HKV, HD = 32, 8, 128
HD2 = HD // 2
NC = 8
HL = H // NC            # 4 local q heads per core
BS = B * S              # 4096 global rows
R = BS // NC            # 512 output rows per core
NRB = BS // 128         # 32 row blocks
NDT = D // 128          # 32 contraction tiles
SCALE = 1.0 / float(np.sqrt(HD))
BF = mybir.dt.bfloat16
F32 = mybir.dt.float32

PROFILE = False         # set by test.py for neuron-profile capture
TMPDIR = None           # set by test.py to keep the trace dir


def _emit(nc, tc, io):
    xTr, wqkvT, woTr, ccssR, trim, onec, iden, onerow, out = io

    cstack = ExitStack()
    bstack = ExitStack()
    with (
        tc.tile_pool(name="ps", bufs=1, space="PSUM") as ps,
        tc.tile_pool(name="cbuf", bufs=1) as cbuf,
        tc.tile_pool(name="cs", bufs=3) as cs,
        tc.tile_pool(name="es", bufs=6) as es,
        tc.tile_pool(name="ts", bufs=8) as ts,
        tc.tile_pool(name="ans", bufs=4) as ans,
        tc.tile_pool(name="rsp", bufs=3) as rsp,
        tc.tile_pool(name="dram", bufs=1, space="DRAM") as dram,
    ):
        # pools released after phase C (attention) to make room for the
        # fp32 output accumulator.
        qbuf = cstack.enter_context(tc.tile_pool(name="qbuf", bufs=1))
        kvbuf = cstack.enter_context(tc.tile_pool(name="kvbuf", bufs=1))
        # pools released after phase B (projection).
        wbuf = bstack.enter_context(tc.tile_pool(name="wbuf", bufs=1))
        xs = bstack.enter_context(tc.tile_pool(name="xs", bufs=3))

        q_sb = qbuf.tile([128, HL * BS], BF, tag="q")     # col = h*4096 + row
        kT_sb = kvbuf.tile([128, BS], BF, tag="k")        # col = row
        v_sb = kvbuf.tile([128, BS], BF, tag="v")         # col = rb*128 + hd

        # x slabs: host-packed so slab rb = [128, 32*128] contiguous; split
        # into two half-slab DMAs on different engines (per-queue DMA BW).
        xt_tiles = {}
        ccss_tiles = {}

        def load_rb(rb):
            xt = xs.tile([128, NDT * 128], BF, tag="x", name=f"xt{rb}")
            nc.sync.dma_start(
                xt[:, 0:2048], xTr[rb * 128: (rb + 1) * 128, 0:2048])
            nc.gpsimd.dma_start(
                xt[:, 2048:4096], xTr[rb * 128: (rb + 1) * 128, 2048:4096])
            xt_tiles[rb] = xt
            cst = cs.tile([128, 512], BF, tag="cc", name=f"cs{rb}")
            nc.gpsimd.dma_start(cst[:], ccssR[:, rb * 512: (rb + 1) * 512])
            ccss_tiles[rb] = cst

        load_rb(0)
        load_rb(1)

        # constants (sync queue, tiny)
        trim_sb = cbuf.tile([128, 128], F32, tag="tm")
        nc.sync.dma_start(trim_sb[:], trim[:])
        onec_sb = cbuf.tile([128, 1], BF, tag="oc")
        nc.sync.dma_start(onec_sb[:], onec[:])
        iden_sb = cbuf.tile([128, 128], BF, tag="idn")
        nc.sync.dma_start(iden_sb[:], iden[:])
        onerow_sb = cbuf.tile([1, 128], F32, tag="orw")
        nc.sync.dma_start(onerow_sb[:], onerow[:])

        # resident QKV weights: col = dt*768 + [0:512 q | 512:640 k | 640:768 v]
        # all on the scalar queue (idle during early phase B).
        w_sb = wbuf.tile([128, NDT * 768], BF, tag="w")
        for dt in range(NDT):
            nc.scalar.dma_start(
                w_sb[:, dt * 768: dt * 768 + 768],
                wqkvT[dt * 128: (dt + 1) * 128, :],
            )

        # Per-head AllToAll buffers: input row block (b*4+ci)*128+hd holds
        # this core's head h attention output for destination rank b*4+ci;
        # output row block i*128+hd holds source core i's head (4i+h) for
        # this core's rows.
        a2a_in = [dram.tile([8 * 128, R], BF, name=f"a2a_in{h}")
                  for h in range(HL)]
        a2a_out = [dram.tile([8 * 128, R], BF, name=f"a2a_out{h}")
                   for h in range(HL)]

        # ---- phase B: QKV projection + RoPE + transposes ----
        def b_rope_tail_q(rb, ps_q):
            cct = ccss_tiles[rb][:, 0:256]
            sst = ccss_tiles[rb][:, 256:512]
            # q rotation, all 4 heads at once via strided APs
            qe = ps_q[:].rearrange("p (h d) -> p h d", d=128)[:, :, 0:HD2]
            qo = ps_q[:].rearrange("p (h d) -> p h d", d=128)[:, :, HD2:HD]
            t1 = ts.tile([128, 256], BF, tag="t")
            t2 = ts.tile([128, 256], BF, tag="t")
            t3 = ts.tile([128, 256], BF, tag="t")
            t4 = ts.tile([128, 256], BF, tag="t")
            nc.vector.tensor_mul(t1[:], qe, cct)
            nc.vector.tensor_mul(t2[:], qo, sst)
            nc.vector.tensor_mul(t3[:], qe, sst)
            nc.vector.tensor_mul(t4[:], qo, cct)
            qrot = ts.tile([128, 512], BF, tag="qr", bufs=4)
            qre = qrot[:].rearrange("p (h d) -> p h d", d=128)[:, :, 0:HD2]
            qro = qrot[:].rearrange("p (h d) -> p h d", d=128)[:, :, HD2:HD]
            nc.vector.tensor_sub(qre, t1[:], t2[:])
            nc.vector.tensor_add(qro, t3[:], t4[:])
            return (qrot,)

        def b_transpose_tail_q(rb, qrot):
            ps_tq = ps.tile([128, 512], BF, tag="at", bufs=2)
            for h in range(HL):
                nc.tensor.transpose(
                    ps_tq[:, h * 128: (h + 1) * 128],
                    qrot[:, h * 128: (h + 1) * 128],
                    iden_sb[:],
                )
            q_dst = (
                q_sb[:]
                .rearrange("p (h r) -> p h r", h=HL)
                [:, :, rb * 128: (rb + 1) * 128]
            )
            nc.vector.tensor_copy(
                q_dst, ps_tq[:].rearrange("p (h r) -> p h r", h=HL)
            )

        def b_rope_tail_kv(rb, ps_kv):
            cct = ccss_tiles[rb][:, 0:256]
            sst = ccss_tiles[rb][:, 256:512]
            ke = ps_kv[:, 0:HD2]
            ko = ps_kv[:, HD2:HD]
            u1 = ts.tile([128, 64], BF, tag="u")
            u2 = ts.tile([128, 64], BF, tag="u")
            u3 = ts.tile([128, 64], BF, tag="u")
            u4 = ts.tile([128, 64], BF, tag="u")
            nc.vector.tensor_mul(u1[:], ke, cct[:, 0:HD2])
            nc.vector.tensor_mul(u2[:], ko, sst[:, 0:HD2])
            nc.vector.tensor_mul(u3[:], ke, sst[:, 0:HD2])
            nc.vector.tensor_mul(u4[:], ko, cct[:, 0:HD2])
            krot = ts.tile([128, 128], BF, tag="kr")
            nc.vector.tensor_sub(krot[:, 0:HD2], u1[:], u2[:])
            nc.vector.tensor_add(krot[:, HD2:HD], u3[:], u4[:])

            # v: plain copy to row-major storage
            nc.scalar.activation(
                v_sb[:, rb * 128: (rb + 1) * 128], ps_kv[:, 128:256],
                mybir.ActivationFunctionType.Copy,
            )
            return (krot,)

        def b_transpose_tail_kv(rb, krot):
            ps_tk = ps.tile([128, 128], BF, tag="rs", bufs=2)
            nc.tensor.transpose(ps_tk[:], krot[:], iden_sb[:])
            nc.vector.tensor_copy(kT_sb[:, rb * 128: (rb + 1) * 128], ps_tk[:])

        pending = None
        rot = None
        for rb in range(NRB):
            if rb + 2 < NRB:
                load_rb(rb + 2)
            ps_q = ps.tile([128, 512], F32, tag="s", bufs=4)  # [rows, 4 q heads]
            ps_kv = ps.tile([128, 256], F32, tag="s", bufs=4)  # [rows, k|v]
            xt = xt_tiles[rb]
            for dt in range(NDT):
                st, sp = dt == 0, dt == NDT - 1
                nc.tensor.matmul(
                    ps_q[:], xt[:, dt * 128: (dt + 1) * 128],
                    w_sb[:, dt * 768: dt * 768 + 512],
                    start=st, stop=sp,
                )
                nc.tensor.matmul(
                    ps_kv[:], xt[:, dt * 128: (dt + 1) * 128],
                    w_sb[:, dt * 768 + 512: dt * 768 + 768],
                    start=st, stop=sp,
                )
                if dt == 2 and pending is not None:
                    rot = (pending[0],) + b_rope_tail_q(pending[0], pending[1]) \
                        + b_rope_tail_kv(pending[0], pending[2])
                    pending = None
                if dt == 12 and rot is not None:
                    b_transpose_tail_q(rot[0], rot[1])
                    b_transpose_tail_kv(rot[0], rot[2])
                    rot = None
            pending = (rb, ps_q, ps_kv)
            del xt_tiles[rb]
        rot = (pending[0],) + b_rope_tail_q(pending[0], pending[1]) \
            + b_rope_tail_kv(pending[0], pending[2])
        b_transpose_tail_q(rot[0], rot[1])
        b_transpose_tail_kv(rot[0], rot[2])

        # release B-only SBUF (weights + x slabs); open D-phase streaming
        # pools on the right side of SBUF.
        bstack.close()
        abuf = tc.alloc_tile_pool(name="abuf", bufs=2, side="right")
        ws = tc.alloc_tile_pool(name="ws", bufs=2, side="right")
        osp = tc.alloc_tile_pool(name="os", bufs=3, side="right")

        # ---- phase C: causal attention, paired interleaved chains ----
        # Each (b, h, ci) is an independent chain; two chains are emitted
        # interleaved so one chain's exp latency hides under the other's
        # matmuls. Pairing ci=0 with ci=3 (and 1 with 2) balances lengths.
        def attn_chain(b, h, ci):
            qbase = h * BS + b * S
            ps_attn = ps.tile([128, 512], F32, tag="at", bufs=2,
                              name=f"pa{b}{h}{ci}")
            ps_rs = ps.tile([1, 512], F32, tag="rs", bufs=2,
                            name=f"pr{b}{h}{ci}")
            jmax = 4 * ci + 3

            def qspan(j):
                q0 = max(j * 128, 512 * ci)
                return q0, 512 * ci + 512 - q0

            def scores(j):
                q0, w = qspan(j)
                kcol = (b * 16 + j) * 128
                ps_s = ps.tile([128, 512], F32, tag="s", bufs=4, name=f"s{j}")
                nc.tensor.matmul(
                    ps_s[:, 0:w],
                    kT_sb[:, kcol: kcol + 128],
                    q_sb[:, qbase + q0: qbase + q0 + w],
                    start=True, stop=True,
                )
                if j // 4 == ci:
                    nc.vector.tensor_add(
                        ps_s[:, 0:128], ps_s[:, 0:128], trim_sb[:]
                    )
                et = es.tile([128, 512], BF, tag="e", name=f"e{j}")
                nc.scalar.activation(
                    et[:, 0:w], ps_s[:, 0:w],
                    mybir.ActivationFunctionType.Exp, scale=SCALE,
                )
                return et

            def pv(j, et):
                q0, w = qspan(j)
                off = q0 - 512 * ci
                kcol = (b * 16 + j) * 128
                nc.tensor.matmul(
                    ps_attn[:, off: off + w],
                    v_sb[:, kcol: kcol + 128],
                    et[:, 0:w],
                    start=(j == 0), stop=(j == jmax),
                )
                nc.tensor.matmul(
                    ps_rs[:, off: off + w],
                    onec_sb[:],
                    et[:, 0:w],
                    start=(j == 0), stop=(j == jmax),
                )

            prev = None
            for j in range(jmax + 1):
                et = scores(j)
                if prev is not None:
                    pv(prev[0], prev[1])
                prev = (j, et)
                yield
            pv(prev[0], prev[1])
            rc = rsp.tile([1, 512], F32, tag="rc")
            nc.vector.reciprocal_approx_fast(rc[:], ps_rs[:])
            bc_sb = rsp.tile([128, 512], F32, tag="bcs")
            nc.gpsimd.partition_broadcast(bc_sb[:], rc[:])
            an = ans.tile([128, 512], BF, tag="an")
            nc.vector.tensor_mul(an[:], ps_attn[:], bc_sb[:])
            blk = (b * 4 + ci) * 128
            nc.sync.dma_start(a2a_in[h][blk: blk + 128, :], an[:])
            yield

        # Head-major worklist; fire head h's AllToAll as soon as its 8
        # chains are done so only the last AllToAll lands after attention.
        def drive(todo):
            todo = list(todo)
            active = []
            while todo or active:
                while len(active) < 2 and todo:
                    active.append(attn_chain(*todo.pop(0)))
                for g in list(active):
                    if next(g, StopIteration) is StopIteration:
                        active.remove(g)

        for h in range(HL):
            drive([(b, h, ci) for b in range(B) for ci in (0, 3, 1, 2)])
            nc.gpsimd.collective_compute(
                "AllToAll",
                mybir.AluOpType.bypass,
                replica_groups=[list(range(NC))],
                ins=[a2a_in[h].opt()],
                outs=[a2a_out[h].opt()],
            )

        # attention buffers dead; reuse their SBUF for the fp32 output
        # accumulator (chunked output projection).
        cstack.close()
        accbuf = tc.alloc_tile_pool(name="accbuf", bufs=1)
        acc = [accbuf.tile([128, D], F32, tag=f"a{rt}", name=f"acc{rt}")
               for rt in range(4)]

        # ---- phase D: output projection, one chunk per head index ----
        # chunk k covers head-tiles ht = 4i+k (head k of each source core);
        # wo slabs are host-packed so slab (k, cg) = [128, 8*512] contiguous.
        for k in range(4):
            at_k = abuf.tile([128, 8 * 512], BF, tag="at", name=f"at{k}")
            nc.gpsimd.dma_start(
                at_k[:].rearrange("p (i c) -> p i c", i=8),
                a2a_out[k][:].rearrange("(i p) c -> p i c", p=128))
            wt = None
            nxt = ws.tile([128, 8 * 512], BF, tag="wo", name=f"wt{k}0")
            nc.scalar.dma_start(
                nxt[:, 0:2048], woTr[(k * 8) * 128: (k * 8 + 1) * 128, 0:2048])
            nc.sync.dma_start(
                nxt[:, 2048:4096],
                woTr[(k * 8) * 128: (k * 8 + 1) * 128, 2048:4096])
            for cg in range(8):
                wt, nxt = nxt, None
                if cg < 7:
                    row = (k * 8 + cg + 1) * 128
                    nxt = ws.tile([128, 8 * 512], BF, tag="wo",
                                  name=f"wt{k}{cg + 1}")
                    nc.scalar.dma_start(nxt[:, 0:2048], woTr[row: row + 128, 0:2048])
                    nc.sync.dma_start(
                        nxt[:, 2048:4096], woTr[row: row + 128, 2048:4096])
                for rt in range(4):
                    po = ps.tile([128, 512], F32, tag="s", bufs=4,
                                 name=f"po{k}{cg}{rt}")
                    for i in range(8):
                        nc.tensor.matmul(
                            po[:],
                            at_k[:, i * 512 + rt * 128: i * 512 + rt * 128 + 128],
                            wt[:, i * 512: (i + 1) * 512],
                            start=(i == 0), stop=(i == 7),
                        )
                    aslice = acc[rt][:, cg * 512: (cg + 1) * 512]
                    if k == 0:
                        nc.vector.tensor_copy(aslice, po[:])
                    elif k < 3:
                        nc.vector.tensor_add(aslice, aslice, po[:])
                    else:
                        ot = osp.tile([128, 512], F32, tag="o")
                        nc.vector.tensor_add(ot[:], aslice, po[:])
                        nc.sync.dma_start(
                            out[rt * 128: (rt + 1) * 128,
                                cg * 512: (cg + 1) * 512], ot[:])
        accbuf.release()
        osp.release()
        ws.release()
        abuf.release()


def _build():
    # NOTE: enable-ldw-opt=true crashes walrus codegen in visitInstLdweights;
    # do not enable.
    nc = bacc.Bacc("TRN2", target_bir_lowering=False, debug=False, num_devices=NC)
    xTr = nc.dram_tensor("xTr", [BS, D], BF, kind="ExternalInput")
    wqkvT = nc.dram_tensor("wqkvT", [D, 768], BF, kind="ExternalInput")
    woTr = nc.dram_tensor("woTr", [D, D], BF, kind="ExternalInput")
    ccssR = nc.dram_tensor("ccssR", [128, NRB * 512], BF, kind="ExternalInput")
    trim = nc.dram_tensor("trim", [128, 128], F32, kind="ExternalInput")
    onec = nc.dram_tensor("onec", [128, 1], BF, kind="ExternalInput")
    iden = nc.dram_tensor("iden", [128, 128], BF, kind="ExternalInput")
    onerow = nc.dram_tensor("onerow", [1, 128], F32, kind="ExternalInput")
    out = nc.dram_tensor("out", [R, D], F32, kind="ExternalOutput")
    with tile.TileContext(nc) as tc:
        _emit(nc, tc, (xTr, wqkvT, woTr, ccssR, trim, onec, iden, onerow, out))
    nc.compile()
    return nc


_NC = None


def kernel(x, wq, wk, wv, wo, freqs_cos, freqs_sin, mask, start_pos):
    global _NC
    if _NC is None:
        _NC = _build()
    nc = _NC
    bf = ml_dtypes.bfloat16

    x = np.asarray(x, dtype=np.float32)
    xT = np.ascontiguousarray(x.reshape(BS, D).T)
    # pack so slab rb = [128 partitions, 32 dt * 128 cols] is contiguous:
    # xTr[rb*128 + p, dt*128 + c] = xT[dt*128 + p, rb*128 + c]
    xTr = np.ascontiguousarray(
        xT.reshape(NDT, 128, NRB, 128).transpose(2, 1, 0, 3).reshape(BS, D)
    ).astype(bf)

    perm = np.concatenate([np.arange(0, HD, 2), np.arange(1, HD, 2)])
    wqTp = np.asarray(wq, np.float32).T.reshape(D, H, HD)[:, :, perm]
    wkTp = np.asarray(wk, np.float32).T.reshape(D, HKV, HD)[:, :, perm]
    wvT = np.asarray(wv, np.float32).T.reshape(D, HKV, HD)
    woT = np.asarray(wo, np.float32).T
    # pack so slab (k, cg) = [128 partitions, 8 i * 512 cols] is contiguous:
    # woTr[(k*8+cg)*128 + p, i*512 + c] = woT[(i*4+k)*128 + p, cg*512 + c]
    woTr = np.ascontiguousarray(
        woT.reshape(8, 4, 128, 8, 512).transpose(1, 3, 2, 0, 4).reshape(D, D)
    ).astype(bf)

    fc = np.asarray(freqs_cos, np.float32)
    fs = np.asarray(freqs_sin, np.float32)
    # row-major RoPE tables per row block, replicated x4 along free axis,
    # cos and sin packed side by side: [128, rb*512 + (0:256 cos|256:512 sin)]
    pos = (np.arange(BS) % S).reshape(NRB, 128)
    ccR = np.tile(fc[pos], (1, 1, 4))          # (NRB, 128, 256)
    ssR = np.tile(fs[pos], (1, 1, 4))
    ccssR = np.concatenate([ccR, ssR], axis=2)  # (NRB, 128, 512)
    ccssR = np.ascontiguousarray(
        ccssR.transpose(1, 0, 2).reshape(128, NRB * 512)
    ).astype(bf)

    trim = np.where(
        np.arange(128)[:, None] > np.arange(128)[None, :], -1e30, 0.0
    ).astype(np.float32)
    onec = np.ones((128, 1), dtype=bf)
    iden = np.eye(128, dtype=bf)
    onerow = np.ones((1, 128), dtype=np.float32)

    in_maps = []
    for c in range(NC):
        wqkv = np.concatenate(
            [
                wqTp[:, 4 * c: 4 * c + 4].reshape(D, 512),
                wkTp[:, c],
                wvT[:, c],
            ],
            axis=1,
        ).astype(bf)
        in_maps.append(
            {
                "xTr": xTr,
                "wqkvT": np.ascontiguousarray(wqkv),
                "woTr": woTr,
                "ccssR": ccssR,
                "trim": trim,
                "onec": onec,
                "iden": iden,
                "onerow": onerow,
            }
        )

    res = bass_utils.run_bass_kernel_spmd(
        nc, in_maps, core_ids=list(range(NC)), trace=PROFILE, tmpdir=TMPDIR
    )
    if PROFILE:
        print(f"HW exec time: {res.exec_time_ns} ns")
        if res.instructions_and_trace is not None:
            print(f"trace: {res.instructions_and_trace[1]}")

    out_full = np.empty((BS, D), dtype=np.float32)
    for c in range(NC):
        out_full[R * c: R * (c + 1)] = res.results[c]["out"]
    return out_full.reshape(B, S, D)


# revision 19
# speedup vs baseline: 1.2103x; 1.0114x over previous
"""Distributed Trainium2 attention kernel (8 NeuronCores).

Strategy: tensor-parallel over heads for QKV projection + attention
(4 query heads + their 1 shared KV head per core, identical causal loop
structure on every core), then AllToAlls switch to row-sharding so each
core computes the output projection for its 512 rows with the full wo.
Host reassembles rows. All matmuls run in bf16 with fp32 PSUM
accumulation; softmax runs unnormalized with the normalization folded in
after the PV matmul (per-head row sums via a ones-matmul).

RoPE is applied in row-major layout via a host-side even/odd column
permutation of wq/wk (rotation becomes contiguous half-block arithmetic),
then q/k are transposed to [head_dim, rows] on the TensorEngine for the
attention matmuls.

Perf structure (the tile scheduler overlaps phases wherever data deps
allow, so emission order mostly sets priorities):
- x / wo / rope tables are host-packed so SBUF tiles fill from 1-2
  contiguous-slab DMAs split across engine queues (per-queue DMA
  bandwidth is only ~90GB/s, and per-tile dma_start issue costs ~630ns).
- QKV weights stream on the scalar queue so x slabs never sit behind them.
- The attention worklist is HEAD-major and each head's AllToAll fires as
  soon as that head's 8 chains are done, so all comm except the last
  1MB AllToAll overlaps attention; the output projection is chunked
  per-head with an fp32 SBUF accumulator so the last AllToAll hides
  under the first chunks' matmuls.
- The softmax normalization tail uses reciprocal_approx_fast (0.7us vs
  3.3us exact), and PSUM is split into per-role rings so a chain's
  scores never wait on the previous chain's tail.
"""

import numpy as np
import ml_dtypes
from contextlib import ExitStack

import concourse.bass as bass
import concourse.mybir as mybir
import concourse.tile as tile
from concourse import bacc
from concourse import bass_utils

B, S, D = 2, 2048, 4096
H, HKV, HD = 32, 8, 128
HD2 = HD // 2
NC = 8
HL = H // NC            # 4 local q heads per core
BS = B * S              # 4096 global rows
R = BS // NC            # 512 output rows per core
NRB = BS // 128         # 32 row blocks
NDT = D // 128          # 32 contraction tiles
SCALE = 1.0 / float(np.sqrt(HD))
BF = mybir.dt.bfloat16
F32 = mybir.dt.float32

PROFILE = False         # set by test.py for neuron-profile capture
TMPDIR = None           # set by test.py to keep the trace dir


def _emit(nc, tc, io):
    xTr, wqkvT, woTr, ccssR, trim, onec, iden, onerow, out = io

    cstack = ExitStack()
    bstack = ExitStack()
    with (
        tc.tile_pool(name="ps", bufs=1, space="PSUM") as ps,
        tc.tile_pool(name="cbuf", bufs=1) as cbuf,
        tc.tile_pool(name="cs", bufs=3) as cs,
        tc.tile_pool(name="es", bufs=6) as es,
        tc.tile_pool(name="ts", bufs=8) as ts,
        tc.tile_pool(name="ans", bufs=4) as ans,
        tc.tile_pool(name="rsp", bufs=3) as rsp,
        tc.tile_pool(name="dram", bufs=1, space="DRAM") as dram,
    ):
        # pools released after phase C (attention) to make room for the
        # fp32 output accumulator.
        qbuf = cstack.enter_context(tc.tile_pool(name="qbuf", bufs=1))
        kvbuf = cstack.enter_context(tc.tile_pool(name="kvbuf", bufs=1))
        # pools released after phase B (projection).
        wbuf = bstack.enter_context(tc.tile_pool(name="wbuf", bufs=1))
        xs = bstack.enter_context(tc.tile_pool(name="xs", bufs=3))

        q_sb = qbuf.tile([128, HL * BS], BF, tag="q")     # col = h*4096 + row
        kT_sb = kvbuf.tile([128, BS], BF, tag="k")        # col = row
        v_sb = kvbuf.tile([128, BS], BF, tag="v")         # col = rb*128 + hd

        # x slabs: host-packed so slab rb = [128, 32*128] contiguous; split
        # into two half-slab DMAs on different engines (per-queue DMA BW).
        xt_tiles = {}
        ccss_tiles = {}

        def load_rb(rb, pieces=2):
            # pieces=4 for the first blocks: smaller DMAs land sooner, so
            # the very first matmuls aren't gated on a full half-slab.
            xt = xs.tile([128, NDT * 128], BF, tag="x", name=f"xt{rb}")
            w = 4096 // pieces
            for p in range(pieces):
                eng = (nc.sync, nc.gpsimd)[p % 2]
                eng.dma_start(
                    xt[:, p * w: (p + 1) * w],
                    xTr[rb * 128: (rb + 1) * 128, p * w: (p + 1) * w])
            xt_tiles[rb] = xt
            cst = cs.tile([128, 512], BF, tag="cc", name=f"cs{rb}")
            nc.gpsimd.dma_start(cst[:], ccssR[:, rb * 512: (rb + 1) * 512])
            ccss_tiles[rb] = cst

        load_rb(0, pieces=4)
        load_rb(1, pieces=4)

        # constants (sync queue, tiny)
        trim_sb = cbuf.tile([128, 128], F32, tag="tm")
        nc.sync.dma_start(trim_sb[:], trim[:])
        onec_sb = cbuf.tile([128, 1], BF, tag="oc")
        nc.sync.dma_start(onec_sb[:], onec[:])
        iden_sb = cbuf.tile([128, 128], BF, tag="idn")
        nc.sync.dma_start(iden_sb[:], iden[:])
        onerow_sb = cbuf.tile([1, 128], F32, tag="orw")
        nc.sync.dma_start(onerow_sb[:], onerow[:])

        # resident QKV weights: col = dt*768 + [0:512 q | 512:640 k | 640:768 v]
        # all on the scalar queue (idle during early phase B).
        w_sb = wbuf.tile([128, NDT * 768], BF, tag="w")
        for dt in range(NDT):
            nc.scalar.dma_start(
                w_sb[:, dt * 768: dt * 768 + 768],
                wqkvT[dt * 128: (dt + 1) * 128, :],
            )

        # Per-head AllToAll buffers: input row block (b*4+ci)*128+hd holds
        # this core's head h attention output for destination rank b*4+ci;
        # output row block i*128+hd holds source core i's head (4i+h) for
        # this core's rows.
        a2a_in = [dram.tile([8 * 128, R], BF, name=f"a2a_in{h}")
                  for h in range(HL)]
        a2a_out = [dram.tile([8 * 128, R], BF, name=f"a2a_out{h}")
                   for h in range(HL)]

        # ---- phase B: QKV projection + RoPE + transposes ----
        def b_rope_tail_q(rb, ps_q):
            cct = ccss_tiles[rb][:, 0:256]
            sst = ccss_tiles[rb][:, 256:512]
            # q rotation, all 4 heads at once via strided APs
            qe = ps_q[:].rearrange("p (h d) -> p h d", d=128)[:, :, 0:HD2]
            qo = ps_q[:].rearrange("p (h d) -> p h d", d=128)[:, :, HD2:HD]
            t1 = ts.tile([128, 256], BF, tag="t")
            t2 = ts.tile([128, 256], BF, tag="t")
            t3 = ts.tile([128, 256], BF, tag="t")
            t4 = ts.tile([128, 256], BF, tag="t")
            nc.vector.tensor_mul(t1[:], qe, cct)
            nc.vector.tensor_mul(t2[:], qo, sst)
            nc.vector.tensor_mul(t3[:], qe, sst)
            nc.vector.tensor_mul(t4[:], qo, cct)
            qrot = ts.tile([128, 512], BF, tag="qr", bufs=4)
            qre = qrot[:].rearrange("p (h d) -> p h d", d=128)[:, :, 0:HD2]
            qro = qrot[:].rearrange("p (h d) -> p h d", d=128)[:, :, HD2:HD]
            nc.vector.tensor_sub(qre, t1[:], t2[:])
            nc.vector.tensor_add(qro, t3[:], t4[:])
            return (qrot,)

        def b_transpose_tail_q(rb, qrot):
            ps_tq = ps.tile([128, 512], BF, tag="at", bufs=2)
            for h in range(HL):
                nc.tensor.transpose(
                    ps_tq[:, h * 128: (h + 1) * 128],
                    qrot[:, h * 128: (h + 1) * 128],
                    iden_sb[:],
                )
            q_dst = (
                q_sb[:]
                .rearrange("p (h r) -> p h r", h=HL)
                [:, :, rb * 128: (rb + 1) * 128]
            )
            nc.vector.tensor_copy(
                q_dst, ps_tq[:].rearrange("p (h r) -> p h r", h=HL)
            )

        def b_rope_tail_kv(rb, ps_kv):
            cct = ccss_tiles[rb][:, 0:256]
            sst = ccss_tiles[rb][:, 256:512]
            ke = ps_kv[:, 0:HD2]
            ko = ps_kv[:, HD2:HD]
            u1 = ts.tile([128, 64], BF, tag="u")
            u2 = ts.tile([128, 64], BF, tag="u")
            u3 = ts.tile([128, 64], BF, tag="u")
            u4 = ts.tile([128, 64], BF, tag="u")
            nc.vector.tensor_mul(u1[:], ke, cct[:, 0:HD2])
            nc.vector.tensor_mul(u2[:], ko, sst[:, 0:HD2])
            nc.vector.tensor_mul(u3[:], ke, sst[:, 0:HD2])
            nc.vector.tensor_mul(u4[:], ko, cct[:, 0:HD2])
            krot = ts.tile([128, 128], BF, tag="kr")
            nc.vector.tensor_sub(krot[:, 0:HD2], u1[:], u2[:])
            nc.vector.tensor_add(krot[:, HD2:HD], u3[:], u4[:])

            # v: plain copy to row-major storage
            nc.scalar.activation(
                v_sb[:, rb * 128: (rb + 1) * 128], ps_kv[:, 128:256],
                mybir.ActivationFunctionType.Copy,
            )
            return (krot,)

        def b_transpose_tail_kv(rb, krot):
            ps_tk = ps.tile([128, 128], BF, tag="rs", bufs=2)
            nc.tensor.transpose(ps_tk[:], krot[:], iden_sb[:])
            nc.vector.tensor_copy(kT_sb[:, rb * 128: (rb + 1) * 128], ps_tk[:])

        pending = None
        rot = None
        for rb in range(NRB):
            if rb + 2 < NRB:
                load_rb(rb + 2)
            ps_q = ps.tile([128, 512], F32, tag="s", bufs=4)  # [rows, 4 q heads]
            ps_kv = ps.tile([128, 256], F32, tag="s", bufs=4)  # [rows, k|v]
            xt = xt_tiles[rb]
            for dt in range(NDT):
                st, sp = dt == 0, dt == NDT - 1
                nc.tensor.matmul(
                    ps_q[:], xt[:, dt * 128: (dt + 1) * 128],
                    w_sb[:, dt * 768: dt * 768 + 512],
                    start=st, stop=sp,
                )
                nc.tensor.matmul(
                    ps_kv[:], xt[:, dt * 128: (dt + 1) * 128],
                    w_sb[:, dt * 768 + 512: dt * 768 + 768],
                    start=st, stop=sp,
                )
                if dt == 2 and pending is not None:
                    rot = (pending[0],) + b_rope_tail_q(pending[0], pending[1]) \
                        + b_rope_tail_kv(pending[0], pending[2])
                    pending = None
                if dt == 12 and rot is not None:
                    b_transpose_tail_q(rot[0], rot[1])
                    b_transpose_tail_kv(rot[0], rot[2])
                    rot = None
            pending = (rb, ps_q, ps_kv)
            del xt_tiles[rb]
        rot = (pending[0],) + b_rope_tail_q(pending[0], pending[1]) \
            + b_rope_tail_kv(pending[0], pending[2])
        b_transpose_tail_q(rot[0], rot[1])
        b_transpose_tail_kv(rot[0], rot[2])

        # release B-only SBUF (weights + x slabs); open D-phase streaming
        # pools on the right side of SBUF.
        bstack.close()
        abuf = tc.alloc_tile_pool(name="abuf", bufs=2, side="right")
        ws = tc.alloc_tile_pool(name="ws", bufs=2, side="right")
        osp = tc.alloc_tile_pool(name="os", bufs=3, side="right")

        # ---- phase C: causal attention, paired interleaved chains ----
        # Each (b, h, ci) is an independent chain; two chains are emitted
        # interleaved so one chain's exp latency hides under the other's
        # matmuls. Pairing ci=0 with ci=3 (and 1 with 2) balances lengths.
        def attn_chain(b, h, ci):
            qbase = h * BS + b * S
            ps_attn = ps.tile([128, 512], F32, tag="at", bufs=2,
                              name=f"pa{b}{h}{ci}")
            ps_rs = ps.tile([1, 512], F32, tag="rs", bufs=2,
                            name=f"pr{b}{h}{ci}")
            jmax = 4 * ci + 3

            def qspan(j):
                q0 = max(j * 128, 512 * ci)
                return q0, 512 * ci + 512 - q0

            def scores(j):
                q0, w = qspan(j)
                kcol = (b * 16 + j) * 128
                ps_s = ps.tile([128, 512], F32, tag="s", bufs=4, name=f"s{j}")
                nc.tensor.matmul(
                    ps_s[:, 0:w],
                    kT_sb[:, kcol: kcol + 128],
                    q_sb[:, qbase + q0: qbase + q0 + w],
                    start=True, stop=True,
                )
                if j // 4 == ci:
                    nc.vector.tensor_add(
                        ps_s[:, 0:128], ps_s[:, 0:128], trim_sb[:]
                    )
                et = es.tile([128, 512], BF, tag="e", name=f"e{j}")
                nc.scalar.activation(
                    et[:, 0:w], ps_s[:, 0:w],
                    mybir.ActivationFunctionType.Exp, scale=SCALE,
                )
                return et

            def pv(j, et):
                q0, w = qspan(j)
                off = q0 - 512 * ci
                kcol = (b * 16 + j) * 128
                nc.tensor.matmul(
                    ps_attn[:, off: off + w],
                    v_sb[:, kcol: kcol + 128],
                    et[:, 0:w],
                    start=(j == 0), stop=(j == jmax),
                )
                nc.tensor.matmul(
                    ps_rs[:, off: off + w],
                    onec_sb[:],
                    et[:, 0:w],
                    start=(j == 0), stop=(j == jmax),
                )

            prev = None
            for j in range(jmax + 1):
                et = scores(j)
                if prev is not None:
                    pv(prev[0], prev[1])
                prev = (j, et)
                yield
            pv(prev[0], prev[1])
            rc = rsp.tile([1, 512], F32, tag="rc")
            nc.vector.reciprocal_approx_fast(rc[:], ps_rs[:])
            bc_sb = rsp.tile([128, 512], F32, tag="bcs")
            nc.gpsimd.partition_broadcast(bc_sb[:], rc[:])
            an = ans.tile([128, 512], BF, tag="an")
            nc.vector.tensor_mul(an[:], ps_attn[:], bc_sb[:])
            blk = (b * 4 + ci) * 128
            nc.sync.dma_start(a2a_in[h][blk: blk + 128, :], an[:])
            yield

        # Head-major worklist; fire head h's AllToAll as soon as its 8
        # chains are done so only the last AllToAll lands after attention.
        def drive(todo):
            todo = list(todo)
            active = []
            while todo or active:
                while len(active) < 2 and todo:
                    active.append(attn_chain(*todo.pop(0)))
                for g in list(active):
                    if next(g, StopIteration) is StopIteration:
                        active.remove(g)

        for h in range(HL):
            drive([(b, h, ci) for b in range(B) for ci in (0, 3, 1, 2)])
            nc.gpsimd.collective_compute(
                "AllToAll",
                mybir.AluOpType.bypass,
                replica_groups=[list(range(NC))],
                ins=[a2a_in[h].opt()],
                outs=[a2a_out[h].opt()],
            )

        # attention buffers dead; reuse their SBUF for the fp32 output
        # accumulator (chunked output projection).
        cstack.close()
        accbuf = tc.alloc_tile_pool(name="accbuf", bufs=1)
        acc = [accbuf.tile([128, D], F32, tag=f"a{rt}", name=f"acc{rt}")
               for rt in range(4)]

        # ---- phase D: output projection, one chunk per head index ----
        # chunk k covers head-tiles ht = 4i+k (head k of each source core);
        # wo slabs are host-packed so slab (k, cg) = [128, 8*512] contiguous.
        for k in range(4):
            at_k = abuf.tile([128, 8 * 512], BF, tag="at", name=f"at{k}")
            nc.gpsimd.dma_start(
                at_k[:].rearrange("p (i c) -> p i c", i=8),
                a2a_out[k][:].rearrange("(i p) c -> p i c", p=128))
            wt = None
            nxt = ws.tile([128, 8 * 512], BF, tag="wo", name=f"wt{k}0")
            nc.scalar.dma_start(
                nxt[:, 0:2048], woTr[(k * 8) * 128: (k * 8 + 1) * 128, 0:2048])
            nc.sync.dma_start(
                nxt[:, 2048:4096],
                woTr[(k * 8) * 128: (k * 8 + 1) * 128, 2048:4096])
            for cg in range(8):
                wt, nxt = nxt, None
                if cg < 7:
                    row = (k * 8 + cg + 1) * 128
                    nxt = ws.tile([128, 8 * 512], BF, tag="wo",
                                  name=f"wt{k}{cg + 1}")
                    nc.scalar.dma_start(nxt[:, 0:2048], woTr[row: row + 128, 0:2048])
                    nc.sync.dma_start(
                        nxt[:, 2048:4096], woTr[row: row + 128, 2048:4096])
                for rt in range(4):
                    po = ps.tile([128, 512], F32, tag="s", bufs=4,
                                 name=f"po{k}{cg}{rt}")
                    for i in range(8):
                        nc.tensor.matmul(
                            po[:],
                            at_k[:, i * 512 + rt * 128: i * 512 + rt * 128 + 128],
                            wt[:, i * 512: (i + 1) * 512],
                            start=(i == 0), stop=(i == 7),
                        )
                    aslice = acc[rt][:, cg * 512: (cg + 1) * 512]
                    if k == 0:
                        nc.vector.tensor_copy(aslice, po[:])
                    elif k < 3:
                        nc.vector.tensor_add(aslice, aslice, po[:])
                    else:
                        ot = osp.tile([128, 512], F32, tag="o")
                        nc.vector.tensor_add(ot[:], aslice, po[:])
                        nc.sync.dma_start(
                            out[rt * 128: (rt + 1) * 128,
                                cg * 512: (cg + 1) * 512], ot[:])
        accbuf.release()
        osp.release()
        ws.release()
        abuf.release()


def _build():
    # NOTE: enable-ldw-opt=true crashes walrus codegen in visitInstLdweights;
    # do not enable.
    nc = bacc.Bacc("TRN2", target_bir_lowering=False, debug=False, num_devices=NC)
    xTr = nc.dram_tensor("xTr", [BS, D], BF, kind="ExternalInput")
    wqkvT = nc.dram_tensor("wqkvT", [D, 768], BF, kind="ExternalInput")
    woTr = nc.dram_tensor("woTr", [D, D], BF, kind="ExternalInput")
    ccssR = nc.dram_tensor("ccssR", [128, NRB * 512], BF, kind="ExternalInput")
    trim = nc.dram_tensor("trim", [128, 128], F32, kind="ExternalInput")
    onec = nc.dram_tensor("onec", [128, 1], BF, kind="ExternalInput")
    iden = nc.dram_tensor("iden", [128, 128], BF, kind="ExternalInput")
    onerow = nc.dram_tensor("onerow", [1, 128], F32, kind="ExternalInput")
    out = nc.dram_tensor("out", [R, D], F32, kind="ExternalOutput")
    with tile.TileContext(nc) as tc:
        _emit(nc, tc, (xTr, wqkvT, woTr, ccssR, trim, onec, iden, onerow, out))
    nc.compile()
    return nc


_NC = None


def kernel(x, wq, wk, wv, wo, freqs_cos, freqs_sin, mask, start_pos):
    global _NC
    if _NC is None:
        _NC = _build()
    nc = _NC
    bf = ml_dtypes.bfloat16

    x = np.asarray(x, dtype=np.float32)
    xT = np.ascontiguousarray(x.reshape(BS, D).T)
    # pack so slab rb = [128 partitions, 32 dt * 128 cols] is contiguous:
    # xTr[rb*128 + p, dt*128 + c] = xT[dt*128 + p, rb*128 + c]
    xTr = np.ascontiguousarray(
        xT.reshape(NDT, 128, NRB, 128).transpose(2, 1, 0, 3).reshape(BS, D)
    ).astype(bf)

    perm = np.concatenate([np.arange(0, HD, 2), np.arange(1, HD, 2)])
    wqTp = np.asarray(wq, np.float32).T.reshape(D, H, HD)[:, :, perm]
    wkTp = np.asarray(wk, np.float32).T.reshape(D, HKV, HD)[:, :, perm]
    wvT = np.asarray(wv, np.float32).T.reshape(D, HKV, HD)
    woT = np.asarray(wo, np.float32).T
    # pack so slab (k, cg) = [128 partitions, 8 i * 512 cols] is contiguous:
    # woTr[(k*8+cg)*128 + p, i*512 + c] = woT[(i*4+k)*128 + p, cg*512 + c]
    woTr = np.ascontiguousarray(
        woT.reshape(8, 4, 128, 8, 512).transpose(1, 3, 2, 0, 4).reshape(D, D)
    ).astype(bf)

    fc = np.asarray(freqs_cos, np.float32)
    fs = np.asarray(freqs_sin, np.float32)
    # row-major RoPE tables per row block, replicated x4 along free axis,
    # cos and sin packed side by side: [128, rb*512 + (0:256 cos|256:512 sin)]
    pos = (np.arange(BS) % S).reshape(NRB, 128)
    ccR = np.tile(fc[pos], (1, 1, 4))          # (NRB, 128, 256)
    ssR = np.tile(fs[pos], (1, 1, 4))
    ccssR = np.concatenate([ccR, ssR], axis=2)  # (NRB, 128, 512)
    ccssR = np.ascontiguousarray(
        ccssR.transpose(1, 0, 2).reshape(128, NRB * 512)
    ).astype(bf)

    trim = np.where(
        np.arange(128)[:, None] > np.arange(128)[None, :], -1e30, 0.0
    ).astype(np.float32)
    onec = np.ones((128, 1), dtype=bf)
    iden = np.eye(128, dtype=bf)
    onerow = np.ones((1, 128), dtype=np.float32)

    in_maps = []
    for c in range(NC):
        wqkv = np.concatenate(
            [
                wqTp[:, 4 * c: 4 * c + 4].reshape(D, 512),
                wkTp[:, c],
                wvT[:, c],
            ],
            axis=1,
        ).astype(bf)
        in_maps.append(
            {
                "xTr": xTr,
                "wqkvT": np.ascontiguousarray(wqkv),
                "woTr": woTr,
                "ccssR": ccssR,
                "trim": trim,
                "onec": onec,
                "iden": iden,
                "onerow": onerow,
            }
        )

    res = bass_utils.run_bass_kernel_spmd(
        nc, in_maps, core_ids=list(range(NC)), trace=PROFILE, tmpdir=TMPDIR
    )
    if PROFILE:
        print(f"HW exec time: {res.exec_time_ns} ns")
        if res.instructions_and_trace is not None:
            print(f"trace: {res.instructions_and_trace[1]}")

    out_full = np.empty((BS, D), dtype=np.float32)
    for c in range(NC):
        out_full[R * c: R * (c + 1)] = res.results[c]["out"]
    return out_full.reshape(B, S, D)


# revision 20
# speedup vs baseline: 1.2250x; 1.0122x over previous
"""Distributed Trainium2 attention kernel (8 NeuronCores).

Strategy: tensor-parallel over heads for QKV projection + attention
(4 query heads + their 1 shared KV head per core, identical causal loop
structure on every core), then AllToAlls switch to row-sharding so each
core computes the output projection for its 512 rows with the full wo.
Host reassembles rows. All matmuls run in bf16 with fp32 PSUM
accumulation; softmax runs unnormalized with the normalization folded in
after the PV matmul (per-head row sums via a ones-matmul).

RoPE is applied in row-major layout via a host-side even/odd column
permutation of wq/wk (rotation becomes contiguous half-block arithmetic),
then q/k are transposed to [head_dim, rows] on the TensorEngine for the
attention matmuls.

Perf structure (the tile scheduler overlaps phases wherever data deps
allow, so emission order mostly sets priorities):
- x / wo / rope tables are host-packed so SBUF tiles fill from 1-2
  contiguous-slab DMAs split across engine queues (per-queue DMA
  bandwidth is only ~90GB/s, and per-tile dma_start issue costs ~630ns).
- QKV weights stream on the scalar queue so x slabs never sit behind them.
- The attention worklist is HEAD-major and each head's AllToAll fires as
  soon as that head's 8 chains are done, so all comm except the last
  1MB AllToAll overlaps attention; the output projection is chunked
  per-head with an fp32 SBUF accumulator so the last AllToAll hides
  under the first chunks' matmuls.
- The softmax normalization tail uses reciprocal_approx_fast (0.7us vs
  3.3us exact), and PSUM is split into per-role rings so a chain's
  scores never wait on the previous chain's tail.
"""

import numpy as np
import ml_dtypes
from contextlib import ExitStack

import concourse.bass as bass
import concourse.mybir as mybir
import concourse.tile as tile
from concourse import bacc
from concourse import bass_utils

B, S, D = 2, 2048, 4096
H, HKV, HD = 32, 8, 128
HD2 = HD // 2
NC = 8
HL = H // NC            # 4 local q heads per core
BS = B * S              # 4096 global rows
R = BS // NC            # 512 output rows per core
NRB = BS // 128         # 32 row blocks
NDT = D // 128          # 32 contraction tiles
SCALE = 1.0 / float(np.sqrt(HD))
BF = mybir.dt.bfloat16
F32 = mybir.dt.float32

PROFILE = False         # set by test.py for neuron-profile capture
TMPDIR = None           # set by test.py to keep the trace dir


def _emit(nc, tc, io):
    xTr, wqkvT, woTr, ccssR, trim, onec, iden, onerow, out = io

    cstack = ExitStack()
    bstack = ExitStack()
    with (
        tc.tile_pool(name="ps", bufs=1, space="PSUM") as ps,
        tc.tile_pool(name="cbuf", bufs=1) as cbuf,
        tc.tile_pool(name="cs", bufs=3) as cs,
        tc.tile_pool(name="es", bufs=6) as es,
        tc.tile_pool(name="ts", bufs=8) as ts,
        tc.tile_pool(name="ans", bufs=4) as ans,
        tc.tile_pool(name="rsp", bufs=3) as rsp,
        tc.tile_pool(name="dram", bufs=1, space="DRAM") as dram,
    ):
        # pools released after phase C (attention) to make room for the
        # fp32 output accumulator.
        qbuf = cstack.enter_context(tc.tile_pool(name="qbuf", bufs=1))
        kvbuf = cstack.enter_context(tc.tile_pool(name="kvbuf", bufs=1))
        # pools released after phase B (projection).
        wbuf = bstack.enter_context(tc.tile_pool(name="wbuf", bufs=1))
        xs = bstack.enter_context(tc.tile_pool(name="xs", bufs=3))

        q_sb = qbuf.tile([128, HL * BS], BF, tag="q")     # col = h*4096 + row
        kT_sb = kvbuf.tile([128, BS], BF, tag="k")        # col = row
        v_sb = kvbuf.tile([128, BS], BF, tag="v")         # col = rb*128 + hd

        # x slabs: host-packed so slab rb = [128, 32*128] contiguous; split
        # into two half-slab DMAs on different engines (per-queue DMA BW).
        xt_tiles = {}
        ccss_tiles = {}

        def load_rb(rb, pieces=2):
            # pieces=4 for the first blocks: smaller DMAs land sooner, so
            # the very first matmuls aren't gated on a full half-slab.
            xt = xs.tile([128, NDT * 128], BF, tag="x", name=f"xt{rb}")
            w = 4096 // pieces
            for p in range(pieces):
                eng = (nc.sync, nc.gpsimd)[p % 2]
                eng.dma_start(
                    xt[:, p * w: (p + 1) * w],
                    xTr[rb * 128: (rb + 1) * 128, p * w: (p + 1) * w])
            xt_tiles[rb] = xt
            cst = cs.tile([128, 512], BF, tag="cc", name=f"cs{rb}")
            nc.gpsimd.dma_start(cst[:], ccssR[:, rb * 512: (rb + 1) * 512])
            ccss_tiles[rb] = cst

        load_rb(0, pieces=4)
        load_rb(1, pieces=4)

        # constants (sync queue, tiny)
        trim_sb = cbuf.tile([128, 128], F32, tag="tm")
        nc.sync.dma_start(trim_sb[:], trim[:])
        onec_sb = cbuf.tile([128, 1], BF, tag="oc")
        nc.sync.dma_start(onec_sb[:], onec[:])
        iden_sb = cbuf.tile([128, 128], BF, tag="idn")
        nc.sync.dma_start(iden_sb[:], iden[:])
        onerow_sb = cbuf.tile([1, 128], F32, tag="orw")
        nc.sync.dma_start(onerow_sb[:], onerow[:])

        # resident QKV weights: col = dt*768 + [0:512 q | 512:640 k | 640:768 v]
        # all on the scalar queue (idle during early phase B).
        w_sb = wbuf.tile([128, NDT * 768], BF, tag="w")
        for dt in range(NDT):
            nc.scalar.dma_start(
                w_sb[:, dt * 768: dt * 768 + 768],
                wqkvT[dt * 128: (dt + 1) * 128, :],
            )

        # Per-head AllToAll buffers: input row block (b*4+ci)*128+hd holds
        # this core's head h attention output for destination rank b*4+ci;
        # output row block i*128+hd holds source core i's head (4i+h) for
        # this core's rows.
        a2a_in = [dram.tile([8 * 128, R], BF, name=f"a2a_in{h}")
                  for h in range(HL)]
        a2a_out = [dram.tile([8 * 128, R], BF, name=f"a2a_out{h}")
                   for h in range(HL)]

        # ---- phase B: QKV projection + RoPE + transposes ----
        def b_rope_tail_q(rb, ps_q):
            cct = ccss_tiles[rb][:, 0:256]
            sst = ccss_tiles[rb][:, 256:512]
            # q rotation, all 4 heads at once via strided APs
            qe = ps_q[:].rearrange("p (h d) -> p h d", d=128)[:, :, 0:HD2]
            qo = ps_q[:].rearrange("p (h d) -> p h d", d=128)[:, :, HD2:HD]
            t1 = ts.tile([128, 256], BF, tag="t")
            t2 = ts.tile([128, 256], BF, tag="t")
            t3 = ts.tile([128, 256], BF, tag="t")
            t4 = ts.tile([128, 256], BF, tag="t")
            nc.vector.tensor_mul(t1[:], qe, cct)
            nc.vector.tensor_mul(t2[:], qo, sst)
            nc.vector.tensor_mul(t3[:], qe, sst)
            nc.vector.tensor_mul(t4[:], qo, cct)
            qrot = ts.tile([128, 512], BF, tag="qr", bufs=4)
            qre = qrot[:].rearrange("p (h d) -> p h d", d=128)[:, :, 0:HD2]
            qro = qrot[:].rearrange("p (h d) -> p h d", d=128)[:, :, HD2:HD]
            nc.vector.tensor_sub(qre, t1[:], t2[:])
            nc.vector.tensor_add(qro, t3[:], t4[:])
            return (qrot,)

        def b_transpose_tail_q(rb, qrot):
            ps_tq = ps.tile([128, 512], BF, tag="at", bufs=2)
            for h in range(HL):
                nc.tensor.transpose(
                    ps_tq[:, h * 128: (h + 1) * 128],
                    qrot[:, h * 128: (h + 1) * 128],
                    iden_sb[:],
                )
            q_dst = (
                q_sb[:]
                .rearrange("p (h r) -> p h r", h=HL)
                [:, :, rb * 128: (rb + 1) * 128]
            )
            nc.vector.tensor_copy(
                q_dst, ps_tq[:].rearrange("p (h r) -> p h r", h=HL)
            )

        def b_rope_tail_kv(rb, ps_kv):
            cct = ccss_tiles[rb][:, 0:256]
            sst = ccss_tiles[rb][:, 256:512]
            ke = ps_kv[:, 0:HD2]
            ko = ps_kv[:, HD2:HD]
            u1 = ts.tile([128, 64], BF, tag="u")
            u2 = ts.tile([128, 64], BF, tag="u")
            u3 = ts.tile([128, 64], BF, tag="u")
            u4 = ts.tile([128, 64], BF, tag="u")
            nc.vector.tensor_mul(u1[:], ke, cct[:, 0:HD2])
            nc.vector.tensor_mul(u2[:], ko, sst[:, 0:HD2])
            nc.vector.tensor_mul(u3[:], ke, sst[:, 0:HD2])
            nc.vector.tensor_mul(u4[:], ko, cct[:, 0:HD2])
            krot = ts.tile([128, 128], BF, tag="kr")
            nc.vector.tensor_sub(krot[:, 0:HD2], u1[:], u2[:])
            nc.vector.tensor_add(krot[:, HD2:HD], u3[:], u4[:])

            # v: plain copy to row-major storage
            nc.scalar.activation(
                v_sb[:, rb * 128: (rb + 1) * 128], ps_kv[:, 128:256],
                mybir.ActivationFunctionType.Copy,
            )
            return (krot,)

        def b_transpose_tail_kv(rb, krot):
            ps_tk = ps.tile([128, 128], BF, tag="rs", bufs=2)
            nc.tensor.transpose(ps_tk[:], krot[:], iden_sb[:])
            nc.vector.tensor_copy(kT_sb[:, rb * 128: (rb + 1) * 128], ps_tk[:])

        pending = None
        rot = None
        for rb in range(NRB):
            if rb + 2 < NRB:
                load_rb(rb + 2)
            ps_q = ps.tile([128, 512], F32, tag="s", bufs=4)  # [rows, 4 q heads]
            ps_kv = ps.tile([128, 256], F32, tag="s", bufs=4)  # [rows, k|v]
            xt = xt_tiles[rb]
            for dt in range(NDT):
                st, sp = dt == 0, dt == NDT - 1
                nc.tensor.matmul(
                    ps_q[:], xt[:, dt * 128: (dt + 1) * 128],
                    w_sb[:, dt * 768: dt * 768 + 512],
                    start=st, stop=sp,
                )
                nc.tensor.matmul(
                    ps_kv[:], xt[:, dt * 128: (dt + 1) * 128],
                    w_sb[:, dt * 768 + 512: dt * 768 + 768],
                    start=st, stop=sp,
                )
                if dt == 2 and pending is not None:
                    rot = (pending[0],) + b_rope_tail_q(pending[0], pending[1]) \
                        + b_rope_tail_kv(pending[0], pending[2])
                    pending = None
                if dt == 12 and rot is not None:
                    b_transpose_tail_q(rot[0], rot[1])
                    b_transpose_tail_kv(rot[0], rot[2])
                    rot = None
            pending = (rb, ps_q, ps_kv)
            del xt_tiles[rb]
        rot = (pending[0],) + b_rope_tail_q(pending[0], pending[1]) \
            + b_rope_tail_kv(pending[0], pending[2])
        b_transpose_tail_q(rot[0], rot[1])
        b_transpose_tail_kv(rot[0], rot[2])

        # release B-only SBUF (weights + x slabs); open D-phase streaming
        # pools on the right side of SBUF.
        bstack.close()
        abuf = tc.alloc_tile_pool(name="abuf", bufs=2, side="right")
        ws = tc.alloc_tile_pool(name="ws", bufs=2, side="right")
        osp = tc.alloc_tile_pool(name="os", bufs=3, side="right")

        # ---- phase C: causal attention, paired interleaved chains ----
        # Each (b, h, ci) is an independent chain; two chains are emitted
        # interleaved so one chain's exp latency hides under the other's
        # matmuls. Pairing ci=0 with ci=3 (and 1 with 2) balances lengths.
        def attn_chain(b, h, ci):
            qbase = h * BS + b * S
            ps_attn = ps.tile([128, 512], F32, tag="at", bufs=2,
                              name=f"pa{b}{h}{ci}")
            ps_rs = ps.tile([1, 512], F32, tag="rs", bufs=2,
                            name=f"pr{b}{h}{ci}")
            jmax = 4 * ci + 3

            def qspan(j):
                q0 = max(j * 128, 512 * ci)
                return q0, 512 * ci + 512 - q0

            def scores(j):
                q0, w = qspan(j)
                kcol = (b * 16 + j) * 128
                ps_s = ps.tile([128, 512], F32, tag="s", bufs=4, name=f"s{j}")
                nc.tensor.matmul(
                    ps_s[:, 0:w],
                    kT_sb[:, kcol: kcol + 128],
                    q_sb[:, qbase + q0: qbase + q0 + w],
                    start=True, stop=True,
                )
                if j // 4 == ci:
                    nc.vector.tensor_add(
                        ps_s[:, 0:128], ps_s[:, 0:128], trim_sb[:]
                    )
                et = es.tile([128, 512], BF, tag="e", name=f"e{j}")
                nc.scalar.activation(
                    et[:, 0:w], ps_s[:, 0:w],
                    mybir.ActivationFunctionType.Exp, scale=SCALE,
                )
                return et

            def pv(j, et):
                q0, w = qspan(j)
                off = q0 - 512 * ci
                kcol = (b * 16 + j) * 128
                nc.tensor.matmul(
                    ps_attn[:, off: off + w],
                    v_sb[:, kcol: kcol + 128],
                    et[:, 0:w],
                    start=(j == 0), stop=(j == jmax),
                )
                nc.tensor.matmul(
                    ps_rs[:, off: off + w],
                    onec_sb[:],
                    et[:, 0:w],
                    start=(j == 0), stop=(j == jmax),
                )

            prev = None
            for j in range(jmax + 1):
                et = scores(j)
                if prev is not None:
                    pv(prev[0], prev[1])
                prev = (j, et)
                yield
            pv(prev[0], prev[1])
            rc = rsp.tile([1, 512], F32, tag="rc")
            nc.vector.reciprocal_approx_fast(rc[:], ps_rs[:])
            bc_sb = rsp.tile([128, 512], F32, tag="bcs")
            nc.gpsimd.partition_broadcast(bc_sb[:], rc[:])
            an = ans.tile([128, 512], BF, tag="an")
            nc.vector.tensor_mul(an[:], ps_attn[:], bc_sb[:])
            blk = (b * 4 + ci) * 128
            nc.sync.dma_start(a2a_in[h][blk: blk + 128, :], an[:])
            yield

        # Head-major worklist; fire head h's AllToAll as soon as its 8
        # chains are done so only the last AllToAll lands after attention.
        def drive(todo):
            todo = list(todo)
            active = []
            while todo or active:
                while len(active) < 2 and todo:
                    active.append(attn_chain(*todo.pop(0)))
                for g in list(active):
                    if next(g, StopIteration) is StopIteration:
                        active.remove(g)

        for h in range(HL):
            drive([(b, h, ci) for b in range(B) for ci in (0, 3, 1, 2)])
            nc.gpsimd.collective_compute(
                "AllToAll",
                mybir.AluOpType.bypass,
                replica_groups=[list(range(NC))],
                ins=[a2a_in[h].opt()],
                outs=[a2a_out[h].opt()],
            )

        # attention buffers dead; reuse their SBUF for the fp32 output
        # accumulator (chunked output projection).
        cstack.close()
        accbuf = tc.alloc_tile_pool(name="accbuf", bufs=1)
        acc = [accbuf.tile([128, D], F32, tag=f"a{rt}", name=f"acc{rt}")
               for rt in range(4)]

        # ---- phase D: output projection, one chunk per head index ----
        # chunk k covers head-tiles ht = 4i+k (head k of each source core);
        # wo slabs are host-packed so slab (k, cg) = [128, 8*512] contiguous.
        def load_wt(k, cg):
            row = (k * 8 + cg) * 128
            wt = ws.tile([128, 8 * 512], BF, tag="wo", bufs=4,
                         name=f"wt{k}{cg}")
            nc.scalar.dma_start(wt[:, 0:2048], woTr[row: row + 128, 0:2048])
            nc.sync.dma_start(wt[:, 2048:4096], woTr[row: row + 128, 2048:4096])
            return wt

        for k in range(4):
            at_k = abuf.tile([128, 8 * 512], BF, tag="at", name=f"at{k}")
            nc.gpsimd.dma_start(
                at_k[:].rearrange("p (i c) -> p i c", i=8),
                a2a_out[k][:].rearrange("(i p) c -> p i c", p=128))
            # column groups processed in pairs so each at-slice stationary
            # feeds TWO N=512 matmuls: the LDWEIGHTS fully hides under the
            # previous matmul (single-matmul stationaries ran LDW-bound).
            nxt = (load_wt(k, 0), load_wt(k, 1))
            for cp in range(4):
                wt0, wt1 = nxt
                if cp < 3:
                    nxt = (load_wt(k, 2 * cp + 2), load_wt(k, 2 * cp + 3))
                for rt in range(4):
                    poa = ps.tile([128, 512], F32, tag="s", bufs=4,
                                  name=f"po{k}{cp}{rt}a")
                    pob = ps.tile([128, 512], F32, tag="s", bufs=4,
                                  name=f"po{k}{cp}{rt}b")
                    for i in range(8):
                        a_sl = at_k[:, i * 512 + rt * 128:
                                    i * 512 + rt * 128 + 128]
                        nc.tensor.matmul(poa[:], a_sl,
                                         wt0[:, i * 512: (i + 1) * 512],
                                         start=(i == 0), stop=(i == 7))
                        nc.tensor.matmul(pob[:], a_sl,
                                         wt1[:, i * 512: (i + 1) * 512],
                                         start=(i == 0), stop=(i == 7))
                    for po, cg in ((poa, 2 * cp), (pob, 2 * cp + 1)):
                        aslice = acc[rt][:, cg * 512: (cg + 1) * 512]
                        if k == 0:
                            nc.vector.tensor_copy(aslice, po[:])
                        elif k < 3:
                            nc.vector.tensor_add(aslice, aslice, po[:])
                        else:
                            ot = osp.tile([128, 512], F32, tag="o")
                            nc.vector.tensor_add(ot[:], aslice, po[:])
                            nc.sync.dma_start(
                                out[rt * 128: (rt + 1) * 128,
                                    cg * 512: (cg + 1) * 512], ot[:])
        accbuf.release()
        osp.release()
        ws.release()
        abuf.release()


def _build():
    # NOTE: enable-ldw-opt=true crashes walrus codegen in visitInstLdweights;
    # do not enable.
    nc = bacc.Bacc("TRN2", target_bir_lowering=False, debug=False, num_devices=NC)
    xTr = nc.dram_tensor("xTr", [BS, D], BF, kind="ExternalInput")
    wqkvT = nc.dram_tensor("wqkvT", [D, 768], BF, kind="ExternalInput")
    woTr = nc.dram_tensor("woTr", [D, D], BF, kind="ExternalInput")
    ccssR = nc.dram_tensor("ccssR", [128, NRB * 512], BF, kind="ExternalInput")
    trim = nc.dram_tensor("trim", [128, 128], F32, kind="ExternalInput")
    onec = nc.dram_tensor("onec", [128, 1], BF, kind="ExternalInput")
    iden = nc.dram_tensor("iden", [128, 128], BF, kind="ExternalInput")
    onerow = nc.dram_tensor("onerow", [1, 128], F32, kind="ExternalInput")
    out = nc.dram_tensor("out", [R, D], F32, kind="ExternalOutput")
    with tile.TileContext(nc) as tc:
        _emit(nc, tc, (xTr, wqkvT, woTr, ccssR, trim, onec, iden, onerow, out))
    nc.compile()
    return nc


_NC = None


def kernel(x, wq, wk, wv, wo, freqs_cos, freqs_sin, mask, start_pos):
    global _NC
    if _NC is None:
        _NC = _build()
    nc = _NC
    bf = ml_dtypes.bfloat16

    x = np.asarray(x, dtype=np.float32)
    xT = np.ascontiguousarray(x.reshape(BS, D).T)
    # pack so slab rb = [128 partitions, 32 dt * 128 cols] is contiguous:
    # xTr[rb*128 + p, dt*128 + c] = xT[dt*128 + p, rb*128 + c]
    xTr = np.ascontiguousarray(
        xT.reshape(NDT, 128, NRB, 128).transpose(2, 1, 0, 3).reshape(BS, D)
    ).astype(bf)

    perm = np.concatenate([np.arange(0, HD, 2), np.arange(1, HD, 2)])
    wqTp = np.asarray(wq, np.float32).T.reshape(D, H, HD)[:, :, perm]
    wkTp = np.asarray(wk, np.float32).T.reshape(D, HKV, HD)[:, :, perm]
    wvT = np.asarray(wv, np.float32).T.reshape(D, HKV, HD)
    woT = np.asarray(wo, np.float32).T
    # pack so slab (k, cg) = [128 partitions, 8 i * 512 cols] is contiguous:
    # woTr[(k*8+cg)*128 + p, i*512 + c] = woT[(i*4+k)*128 + p, cg*512 + c]
    woTr = np.ascontiguousarray(
        woT.reshape(8, 4, 128, 8, 512).transpose(1, 3, 2, 0, 4).reshape(D, D)
    ).astype(bf)

    fc = np.asarray(freqs_cos, np.float32)
    fs = np.asarray(freqs_sin, np.float32)
    # row-major RoPE tables per row block, replicated x4 along free axis,
    # cos and sin packed side by side: [128, rb*512 + (0:256 cos|256:512 sin)]
    pos = (np.arange(BS) % S).reshape(NRB, 128)
    ccR = np.tile(fc[pos], (1, 1, 4))          # (NRB, 128, 256)
    ssR = np.tile(fs[pos], (1, 1, 4))
    ccssR = np.concatenate([ccR, ssR], axis=2)  # (NRB, 128, 512)
    ccssR = np.ascontiguousarray(
        ccssR.transpose(1, 0, 2).reshape(128, NRB * 512)
    ).astype(bf)

    trim = np.where(
        np.arange(128)[:, None] > np.arange(128)[None, :], -1e30, 0.0
    ).astype(np.float32)
    onec = np.ones((128, 1), dtype=bf)
    iden = np.eye(128, dtype=bf)
    onerow = np.ones((1, 128), dtype=np.float32)

    in_maps = []
    for c in range(NC):
        wqkv = np.concatenate(
            [
                wqTp[:, 4 * c: 4 * c + 4].reshape(D, 512),
                wkTp[:, c],
                wvT[:, c],
            ],
            axis=1,
        ).astype(bf)
        in_maps.append(
            {
                "xTr": xTr,
                "wqkvT": np.ascontiguousarray(wqkv),
                "woTr": woTr,
                "ccssR": ccssR,
                "trim": trim,
                "onec": onec,
                "iden": iden,
                "onerow": onerow,
            }
        )

    res = bass_utils.run_bass_kernel_spmd(
        nc, in_maps, core_ids=list(range(NC)), trace=PROFILE, tmpdir=TMPDIR
    )
    if PROFILE:
        print(f"HW exec time: {res.exec_time_ns} ns")
        if res.instructions_and_trace is not None:
            print(f"trace: {res.instructions_and_trace[1]}")

    out_full = np.empty((BS, D), dtype=np.float32)
    for c in range(NC):
        out_full[R * c: R * (c + 1)] = res.results[c]["out"]
    return out_full.reshape(B, S, D)
